# revision 20
# baseline (speedup 1.0000x reference)
"""Trainium2 Bass kernel for a GPT-style transformer block (B=2, T=2048,
C=1024, 16 heads, MLP 4x), sharded across 8 NeuronCores.

Sharding v2: attention is head-sharded (core 4b+j owns batch b, heads
[4j,4j+4) over ALL 2048 tokens -> exact causal tiling, no kv exchange);
the proj partial output (token-major) is summed+resharded by two chunked
bf16 ReduceScatters (tokens [0,1024) and [1024,2048)), each core receiving
256 tokens per chunk; LN2+MLP run token-sharded on the core's 512 tokens.
The RS output arrives token-major and is flipped back to channel-major by
dma_start_transpose (verified: out[p,s,t] = in[t, 128*s+p]).

Host precomputes LN1, folds LN scale/shift and 1/sqrt(D) into weights,
pre-transposes everything. Residual stays f32 on the output path; matmuls
bf16 with f32 PSUM.
"""
import numpy as np
import ml_dtypes

import concourse.bass as bass
import concourse.mybir as mybir
import concourse.tile as tile
import concourse.bacc as bacc
from concourse.bass_utils import run_bass_kernel_spmd

BF = ml_dtypes.bfloat16
P = 128
B, T, C, H, D, F = 2, 2048, 1024, 16, 64, 4096
NCT = C // P          # 8   c-tiles
NFT = F // P          # 32  f-tiles
NKT = T // P          # 16  kv tiles
EPS = 1e-5
f32 = mybir.dt.float32
bf16 = mybir.dt.bfloat16
AF = mybir.ActivationFunctionType

_CACHED_NC = None


def _build_nc():
    nc = bacc.Bacc("TRN2", target_bir_lowering=False, debug=False)
    d = {}
    for name, shape, dt in [
        ("gT", [C, T], bf16),
        ("WqT", [C, 256], bf16), ("WkT", [C, 256], bf16), ("WvT", [C, 256], bf16),
        ("WpT", [256, C], bf16), ("WupT", [C, F], bf16), ("WdownT", [F, C], bf16),
        ("xbT", [C, 512], bf16),
        ("bq", [P, 2], f32), ("bk", [P, 2], f32), ("brep", [P, 256], bf16),
        ("bup", [P, 32], f32), ("bdown", [P, 8], f32),
        ("maskA", [P, 1024], bf16), ("maskB", [P, 1024], bf16),
    ]:
        d[name] = nc.dram_tensor(name, shape, dt, kind="ExternalInput").ap()
    d["OUT"] = nc.dram_tensor("OUT", [C, 512], f32, kind="ExternalOutput").ap()

    with tile.TileContext(nc) as tc:
        _emit(nc, tc, d)
    nc.compile()
    return nc


def _emit(nc, tc, d):
    import os
    from contextlib import ExitStack

    with ExitStack() as ctx:
        # ---------------- long-lived pools ----------------
        cpool = ctx.enter_context(tc.tile_pool(name="cpool", bufs=1))
        wpool = ctx.enter_context(tc.tile_pool(name="wpool", bufs=1))
        lnp = ctx.enter_context(tc.tile_pool(name="lnp", bufs=1))
        wdp = ctx.enter_context(tc.tile_pool(name="wdp", bufs=1))
        dramp = ctx.enter_context(tc.tile_pool(name="dramp", bufs=1, space="DRAM"))

        attnT = cpool.tile([P, 2, T], bf16, name="attnT")      # 8KB/part
        xbT = cpool.tile([P, 8, 512], bf16, name="xbT")        # 8KB
        x1T = cpool.tile([P, 8, 512], bf16, name="x1T")        # 8KB
        maskA = cpool.tile([P, 1024], bf16, name="maskA")
        maskB = cpool.tile([P, 1024], bf16, name="maskB")
        bq = cpool.tile([P, 2], f32, name="bq")
        bk = cpool.tile([P, 2], f32, name="bk")
        brep = cpool.tile([P, 256], bf16, name="brep")
        bup = cpool.tile([P, 32], f32, name="bup")
        bdown = cpool.tile([P, 8], f32, name="bdown")
        epsT = cpool.tile([P, 1], f32, name="epsT")
        onesb = cpool.tile([P, P], bf16, name="onesb")

        wup = wpool.tile([P, NCT, F], bf16, name="wup")        # 64KB
        wdown_a = wdp.tile([P, 8, C], bf16, name="wdown_a")    # 16KB

        # LN2 working tiles (outlive phase 1 so chunk-0 LN2 can interleave
        # with the attention tail)
        x1g_t = lnp.tile([P, 8, 256], bf16, name="x1g")
        x1g = [x1g_t, x1g_t]
        sqb = lnp.tile([P, 8, 256], bf16, name="sqb")
        g2 = [lnp.tile([P, 8, 256], bf16, name=f"g2{c}") for c in range(2)]
        mu = lnp.tile([P, 256], f32, name="mu")
        e2 = lnp.tile([P, 256], f32, name="e2")
        musq = lnp.tile([P, 256], f32, name="musq")
        std = lnp.tile([P, 256], f32, name="std")

        rs_in = [dramp.tile([1024, C], bf16, name=f"rs_in{c}") for c in range(2)]
        rs_out = [dramp.tile([256, C], bf16, name=f"rs_out{c}") for c in range(2)]

        def emit_ln2_head(c, psum_pool):
            """transpose + residual add; stats; g2 — emitted inside phase 1
            for c=0 (interleaved with attention) and in phase 2 for c=1."""
            csl = slice(c * 256, (c + 1) * 256)
            nc.sync.dma_start_transpose(x1g[c][:], rs_out[c][:])
            with nc.allow_low_precision(reason="residual bf16"):
                nc.vector.tensor_add(x1T[:, :, csl], x1g[c][:], xbT[:, :, csl])
            nc.scalar.activation(sqb[:], x1T[:, :, csl], AF.Square)
            psmu = psum_pool.tile([P, 1024], f32, name=f"psmu{c}", tag="sc")
            pssq = psum_pool.tile([P, 1024], f32, name=f"pssq{c}", tag="sc")
            for ct in range(NCT):
                nc.tensor.matmul(psmu[:, 0:256], onesb[:], x1T[:, ct, csl],
                                 start=(ct == 0), stop=(ct == NCT - 1))
            for ct in range(NCT):
                nc.tensor.matmul(pssq[:, 0:256], onesb[:], sqb[:, ct, :],
                                 start=(ct == 0), stop=(ct == NCT - 1))
            nc.scalar.mul(mu[:], psmu[:, 0:256], 1.0 / C)
            nc.scalar.mul(e2[:], pssq[:, 0:256], 1.0 / C)
            nc.scalar.activation(musq[:], mu[:], AF.Square)
            nc.vector.tensor_sub(e2[:], e2[:], musq[:])
            lg = lnp.tile([P, 256], f32, name="lg") if not hasattr(
                emit_ln2_head, "_lg") else emit_ln2_head._lg
            emit_ln2_head._lg = lg
            nc.scalar.activation(lg[:], e2[:], AF.Ln, bias=epsT[:])
            nc.scalar.activation(std[:], lg[:], AF.Exp, scale=-0.5)
            with nc.allow_low_precision(reason="ln2 bf16"):
                for ct in range(NCT):
                    nc.vector.tensor_sub(sqb[:, ct, :], x1T[:, ct, csl], mu[:])
                    nc.vector.tensor_mul(g2[c][:, ct, :], sqb[:, ct, :], std[:])

        # =========== phase 1: QKV + attention + proj ===========
        with tc.tile_pool(name="p1", bufs=1) as p1:
            qT = p1.tile([P, 2, T], bf16, name="qT")           # 8KB
            kT = p1.tile([P, 2, T], bf16, name="kT")           # 8KB
            v_aug = p1.tile([P, 4, NKT * 65], bf16, name="v_aug")   # 8.3KB
            wp = p1.tile([P, 2, C], bf16, name="wp")           # 4KB
            v4 = v_aug[:].rearrange("p h (k e) -> p h k e", e=65)
            wusrc = d["WupT"].rearrange("(ct p) f -> p ct f", p=P)
            wdsrc = d["WdownT"].rearrange("(cf p) o -> p cf o", p=P)

            # ---- QKV projections ----
            with tc.tile_pool(name="gp", bufs=1) as gp, \
                 tc.tile_pool(name="qkps", bufs=3, space="PSUM") as qkps:
                gT = gp.tile([P, NCT, T], bf16, name="gT")     # 16KB
                wq = gp.tile([P, NCT, 256], bf16, name="wq")
                wk = gp.tile([P, NCT, 256], bf16, name="wk")
                wv = gp.tile([P, NCT, 256], bf16, name="wv")
                gsrc = d["gT"].rearrange("(ct p) t -> p ct t", p=P)
                # first compute gate: gT chunk 0 + wk -> order those first
                nc.sync.dma_start(gT[:, :, 0:512], gsrc[:, :, 0:512])
                for w, key in [(wk, "WkT"), (wq, "WqT"), (wv, "WvT")]:
                    nc.sync.dma_start(w[:],
                                      d[key].rearrange("(ct p) o -> p ct o", p=P))
                for tch in range(1, 4):
                    nc.sync.dma_start(gT[:, :, tch * 512:(tch + 1) * 512],
                                      gsrc[:, :, tch * 512:(tch + 1) * 512])
                for t, key in [(bq, "bq"), (bk, "bk"), (brep, "brep"),
                               (bup, "bup"), (bdown, "bdown"),
                               (maskA, "maskA"), (maskB, "maskB")]:
                    nc.sync.dma_start(t[:], d[key])
                nc.vector.memset(epsT[:], EPS)
                nc.vector.memset(onesb[:], 1.0)
                nc.vector.memset(v4[:, :, :, 64:65], 1.0)
                nc.sync.dma_start(wp[:],
                                  d["WpT"].rearrange("(ct p) o -> p ct o", p=P))
                nc.sync.dma_start(xbT[:],
                                  d["xbT"].rearrange("(ot p) t -> p ot t", p=P))
                nc.sync.dma_start(wup[:], wusrc)
                nc.sync.dma_start(wdown_a[:], wdsrc[:, 0:8, :])

                for tch in range(4):
                    tsl = slice(tch * 512, (tch + 1) * 512)
                    for w, dst, b in [(wk, kT, bk), (wq, qT, bq)]:
                        for ot in range(2):
                            pq = qkps.tile([P, 512], f32, name="pq", tag="qk")
                            for ct in range(NCT):
                                nc.tensor.matmul(
                                    pq[:], w[:, ct, ot * P:(ot + 1) * P],
                                    gT[:, ct, tsl],
                                    start=(ct == 0), stop=(ct == NCT - 1))
                            nc.scalar.add(dst[:, ot, tsl], pq[:], b[:, ot:ot + 1])
                    for tt in range(4 * tch, 4 * tch + 4):
                        pv = qkps.tile([P, 256], f32, name="pv", tag="qk")
                        for ct in range(NCT):
                            nc.tensor.matmul(
                                pv[:], gT[:, ct, tt * P:(tt + 1) * P],
                                wv[:, ct, :],
                                start=(ct == 0), stop=(ct == NCT - 1))
                        nc.vector.tensor_add(
                            v4[:, :, tt, 0:64],
                            pv[:].rearrange("p (h dd) -> p h dd", dd=64),
                            brep[:].rearrange("p (h dd) -> p h dd", dd=64))

            # ---- attention (software-pipelined) + proj chunks ----
            with tc.tile_pool(name="expp", bufs=3) as expp, \
                 tc.tile_pool(name="drp", bufs=2) as drp, \
                 tc.tile_pool(name="rscp", bufs=2) as rscp, \
                 tc.tile_pool(name="scps", bufs=2, space="PSUM") as scps, \
                 tc.tile_pool(name="avps", bufs=2, space="PSUM") as avps:

                def emit_av(av, ex, k, last):
                    for h in range(4):
                        colo = (h % 2) * 512 + (h // 2) * 256
                        nc.tensor.matmul(
                            av[0:65, colo:colo + 256],
                            v_aug[:, h, k * 65:k * 65 + 65],
                            ex[:, colo:colo + 256],
                            start=(k == 0 and h < 2), stop=last,
                            skip_group_check=True)

                def emit_proj(c, tt):
                    # proj partial, token-major: tokens [c*1024+tt*128, +128)
                    pp = scps.tile([P, 1024], f32, name=f"pp{c}_{tt}", tag="sc")
                    for oc in range(2):
                        for ct in range(2):
                            nc.tensor.matmul(
                                pp[:, oc * 512:(oc + 1) * 512],
                                attnT[:, ct, c * 1024 + tt * P:
                                      c * 1024 + (tt + 1) * P],
                                wp[:, ct, oc * 512:(oc + 1) * 512],
                                start=(ct == 0), stop=(ct == 1))
                    rsct = rscp.tile([P, C], bf16, name=f"rsc{c}_{tt}",
                                     tag="rsc")
                    nc.vector.tensor_copy(rsct[:], pp[:])
                    nc.sync.dma_start(rs_in[c][tt * P:(tt + 1) * P, :], rsct[:])

                def send_rs(c):
                    nc.gpsimd.collective_compute(
                        "ReduceScatter", mybir.AluOpType.add,
                        ins=[rs_in[c].opt()], outs=[rs_out[c].opt()],
                        replica_groups=[[0, 1, 2, 3], [4, 5, 6, 7]])

                def emit_epilogue(qg, av):
                    # normalize via reciprocal + K=1 broadcast matmul
                    qsl = slice(qg * 256, (qg + 1) * 256)
                    avsb = drp.tile([P, 1024], bf16, name=f"avsb{qg}",
                                    tag="avsb")
                    nc.vector.tensor_copy(avsb[0:65, :], av[0:65, :])
                    denr = drp.tile([1, 1024], bf16, name=f"denr{qg}",
                                    tag="denr")
                    with nc.allow_low_precision(reason="softmax denom bf16"):
                        nc.vector.reciprocal(denr[:], avsb[64:65, :])
                    for h in range(4):
                        colo = (h % 2) * 512 + (h // 2) * 256
                        nc.tensor.matmul(
                            av[64:128, colo:colo + 256],
                            onesb[0:1, 0:64], denr[0:1, colo:colo + 256],
                            start=True, stop=True, skip_group_check=True)
                    for h in range(4):
                        hb = (h % 2) * 64
                        colo = (h % 2) * 512 + (h // 2) * 256
                        nc.vector.tensor_mul(
                            attnT[hb:hb + 64, h // 2, qsl],
                            avsb[0:64, colo:colo + 256],
                            av[64:128, colo:colo + 256])

                pending = None
                for qg in range(8):
                    K = 2 * qg + 2
                    qsl = slice(qg * 256, (qg + 1) * 256)
                    av = avps.tile([P, 1024], f32, name=f"av{qg}", tag="av")
                    if qg == 6:
                        emit_ln2_head(0, scps)
                    prev = None
                    for k in range(K):
                        sc = scps.tile([P, 1024], f32, name=f"sc{qg}_{k}",
                                       tag="sc")
                        for h in range(4):
                            hb = (h % 2) * 64
                            colo = (h % 2) * 512 + (h // 2) * 256
                            nc.tensor.matmul(
                                sc[:, colo:colo + 256],
                                kT[hb:hb + 64, h // 2, k * P:(k + 1) * P],
                                qT[hb:hb + 64, h // 2, qsl],
                                start=True, stop=True)
                        ex = expp.tile([P, 1024], bf16, name=f"ex{qg}_{k}",
                                       tag="ex")
                        nc.scalar.activation(ex[:], sc[:], AF.Exp)
                        if k == 2 * qg:
                            nc.vector.tensor_mul(ex[:], ex[:], maskA[:])
                        elif k == 2 * qg + 1:
                            nc.vector.tensor_mul(ex[:], ex[:], maskB[:])
                        if k == 2 and pending is not None:
                            emit_epilogue(*pending)   # prev qg, mid-stream
                            pending = None
                        if prev is not None:
                            emit_av(av, prev[0], prev[1], False)
                        prev = (ex, k)
                    emit_av(av, prev[0], prev[1], True)
                    if qg <= 3:
                        emit_epilogue(qg, av)
                    else:
                        pending = (qg, av)
                    # proj: chunk0 complete after qg3 (eager epilogues);
                    # chunk1 spreads behind the pipelined epilogues
                    if qg == 3:
                        for tt in range(8):
                            emit_proj(0, tt)
                        send_rs(0)
                    elif qg >= 5:
                        for tt in range(2 * (qg - 5), 2 * (qg - 5) + 2):
                            emit_proj(1, tt)
                emit_epilogue(*pending)
                for tt in range(6, 8):
                    emit_proj(1, tt)
                send_rs(1)

        # =========== phase 2: MLP, per 256-token chunk ===========
        with tc.tile_pool(name="mlp", bufs=1) as mlp, \
             tc.tile_pool(name="outp", bufs=1) as outp, \
             tc.tile_pool(name="mps", bufs=2, space="PSUM") as mps, \
             tc.tile_pool(name="ups", bufs=2, space="PSUM") as ups, \
             tc.tile_pool(name="dps", bufs=2, space="PSUM") as dps:
            outdst = d["OUT"].rearrange("(ot p) t -> p ot t", p=P)
            wdown_b = mlp.tile([P, 24, C], bf16, name="wdown_b")  # 48KB
            nc.sync.dma_start(wdown_b[:], wdsrc[:, 8:32, :])
            for c in range(2):
                csl = slice(c * 256, (c + 1) * 256)
                # ---- up + gelu ----
                hT = mlp.tile([P, NFT, 256], bf16, name=f"hT{c}", tag="hT")
                for fg in range(NFT // 2):
                    pu = ups.tile([P, 512], f32, name=f"pu{c}_{fg}", tag="pu")
                    for sub in range(2):
                        ft = fg * 2 + sub
                        for ct in range(NCT):
                            nc.tensor.matmul(
                                pu[:, sub * 256:(sub + 1) * 256],
                                wup[:, ct, ft * P:(ft + 1) * P],
                                g2[c][:, ct, :],
                                start=(ct == 0 and sub == 0),
                                stop=(ct == NCT - 1),
                                skip_group_check=True)
                    for sub in range(2):
                        ft = fg * 2 + sub
                        nc.scalar.activation(
                            hT[:, ft, :], pu[:, sub * 256:(sub + 1) * 256],
                            AF.Gelu, bias=bup[:, ft:ft + 1])
                if c == 0:
                    emit_ln2_head(1, mps)
                # ---- down + bias + residual ----
                for ot in range(8):
                    pd = dps.tile([P, 256], f32, name=f"pd{c}_{ot}", tag="pd")
                    for cf in range(NFT):
                        wd, ci = (wdown_a, cf) if cf < 8 else (wdown_b, cf - 8)
                        nc.tensor.matmul(pd[:], wd[:, ci, ot * P:(ot + 1) * P],
                                         hT[:, cf, :],
                                         start=(cf == 0), stop=(cf == NFT - 1))
                    td = outp.tile([P, 256], f32, name=f"td{c}_{ot}", tag="td",
                                   bufs=2)
                    nc.scalar.add(td[:], pd[:], bdown[:, ot:ot + 1])
                    outO = outp.tile([P, 256], f32, name=f"outO{c}_{ot}",
                                     tag="outO", bufs=2)
                    nc.vector.tensor_add(outO[:], td[:], x1T[:, ot, csl])
                    nc.sync.dma_start(outdst[:, ot, csl], outO[:])


def _prep_inputs(x, ln1_w, ln1_b, c_attn_w, c_attn_b, c_proj_w, c_proj_b,
                 ln2_w, ln2_b, up_w, up_b, down_w, down_b):
    """Host-side preprocessing -> list of 8 per-core input dicts."""
    x = np.asarray(x, np.float32)
    f64 = np.float64
    mu = x.mean(-1, keepdims=True, dtype=f64)
    var = np.asarray(x, f64).var(-1, keepdims=True)
    g = ((x - mu) / np.sqrt(var + EPS)).astype(np.float32)     # [B, T, C]

    ln1_w = np.asarray(ln1_w, np.float32); ln1_b = np.asarray(ln1_b, np.float32)
    ln2_w = np.asarray(ln2_w, np.float32); ln2_b = np.asarray(ln2_b, np.float32)
    c_attn_w = np.asarray(c_attn_w, np.float32)
    c_attn_b = np.asarray(c_attn_b, np.float32)
    c_proj_w = np.asarray(c_proj_w, np.float32)
    c_proj_b = np.asarray(c_proj_b, np.float32)
    up_w = np.asarray(up_w, np.float32); up_b = np.asarray(up_b, np.float32)
    down_w = np.asarray(down_w, np.float32)
    down_b = np.asarray(down_b, np.float32)

    Wa = c_attn_w * ln1_w[None, :]
    ba = c_attn_b + c_attn_w @ ln1_b
    Wq, Wk, Wv = Wa[:C], Wa[C:2 * C], Wa[2 * C:]
    bqv, bkv, bvv = ba[:C], ba[C:2 * C], ba[2 * C:]
    s = 1.0 / np.sqrt(D)
    Wq = Wq * s; bqv = bqv * s

    Wup = up_w * ln2_w[None, :]
    bupv = up_b + up_w @ ln2_b

    def b2t(v, n):   # per-partition bias layout [128, n]
        return np.ascontiguousarray(v.reshape(n, P).T.astype(np.float32))

    # diag-tile masks [128 kv, 256 q] tiled x4 heads
    tk = np.arange(P)[:, None]
    tq = np.arange(P)[None, :]
    mA = (tk <= tq).astype(np.float32)
    blockA = np.concatenate([mA, np.ones((P, P), np.float32)], axis=1)
    blockB = np.concatenate([np.zeros((P, P), np.float32), mA], axis=1)
    maskA = np.tile(blockA, (1, 4)).astype(BF)
    maskB = np.tile(blockB, (1, 4)).astype(BF)

    shared = {
        "WupT": np.ascontiguousarray(Wup.T).astype(BF),
        "WdownT": np.ascontiguousarray(down_w.T).astype(BF),
        "bup": b2t(bupv, 32), "bdown": b2t(down_b, 8),
        "maskA": maskA, "maskB": maskB,
    }

    xb = x + c_proj_b[None, None, :]
    in_maps = []
    for core in range(8):
        b, j = core // 4, core % 4
        hsl = slice(256 * j, 256 * j + 256)
        m = dict(shared)
        m["gT"] = np.ascontiguousarray(g[b].T).astype(BF)
        m["WqT"] = np.ascontiguousarray(Wq[hsl].T).astype(BF)
        m["WkT"] = np.ascontiguousarray(Wk[hsl].T).astype(BF)
        m["WvT"] = np.ascontiguousarray(Wv[hsl].T).astype(BF)
        m["WpT"] = np.ascontiguousarray(c_proj_w[:, hsl].T).astype(BF)
        m["bq"] = b2t(bqv[hsl], 2)
        m["bk"] = b2t(bkv[hsl], 2)
        m["brep"] = np.broadcast_to(bvv[hsl].astype(BF), (P, 256)).copy()
        cols = np.r_[256 * j:256 * j + 256, 1024 + 256 * j:1024 + 256 * j + 256]
        m["xbT"] = np.ascontiguousarray(xb[b].T[:, cols]).astype(BF)
        in_maps.append(m)
    return in_maps


def kernel(**inputs):
    global _CACHED_NC
    if _CACHED_NC is None:
        _CACHED_NC = _build_nc()
    nc = _CACHED_NC
    in_maps = _prep_inputs(**inputs)
    try:
        res = run_bass_kernel_spmd(nc, in_maps, list(range(8)))
    except Exception:
        res = run_bass_kernel_spmd(nc, in_maps, list(range(8)))
    out = np.empty((B, T, C), np.float32)
    for core in range(8):
        o = res.results[core]["OUT"]                # [C, 512]
        b, j = core // 4, core % 4
        out[b, 256 * j:256 * j + 256, :] = o[:, 0:256].T
        out[b, 1024 + 256 * j:1024 + 256 * j + 256, :] = o[:, 256:512].T
    return out
